# revision 37
# baseline (speedup 1.0000x reference)
"""Causal self-attention (B=1, T=4096, D=1024, H=16, HD=64) on 8 trn2 NeuronCores.

Sharding: tensor-parallel over heads (2 heads per core) for QKV + attention.
The output projection is COLUMN-sharded (core c computes out[:, c*128:(c+1)*128])
so the re-shard collective can be split into one small AllGather per tq block,
each fully overlapped with the remaining attention compute; the projection for
block b runs as PE filler during block b+2.  Only the last block's gather +
projection (~2% of work) sits on the critical-path tail, vs. a monolithic
end-of-kernel AllToAll + projection in the row-sharded layout.

Matmul layout notes (PE computes out = lhsT.T @ rhs, contraction on partitions):
 - host feeds x transposed (xT [D, T]) so QKV needs no on-chip transposes.
 - S^T tiles [tk, tq] are computed (not S) so the PV matmul can consume
   exp(S^T) directly as the moving operand with V in natural [tk, hd] layout.
 - a ones-column appended to V makes row 64 of the PV accumulator the
   softmax denominator (no extra reduction pass).
 - softmax max-subtraction is skipped: scores are ~N(0,1) (|s| < ~10), and
   a constant shift cancels exactly in softmax, so exp is safe in fp32.
 - causal masking via a DVE multiply with a sliced triangular bf16 mask
   (cheaper than burning PE columns accumulating -1e9); diagonal-chunk QK
   matmuls and exps only cover the causally-reachable column range.
 - attention inner loop is software-pipelined one chunk per iteration
   (QK(ci+2) | exp(ci+1) | PV(ci)) so the PE never waits on the Scalar
   engine's exp chain; QKV projections for the next block, normalization of
   the previous block, and the output projection are interleaved as PE filler.
"""

import math
import sys
from contextlib import ExitStack

sys.path.insert(0, "/opt/trn_rl_repo")

import ml_dtypes
import numpy as np

import concourse.bass as bass  # noqa: F401  (bass types used via tile/bacc)
import concourse.mybir as mybir
import concourse.tile as tile
from concourse import bacc
from concourse.bass_utils import run_bass_kernel_spmd

B, T, D, H, HD = 1, 4096, 1024, 16, 64
NCORES = 8
HPC = H // NCORES          # heads per core = 2
E = HPC * HD               # per-core head width = 128
TQ = 512                   # tq block width
NB = T // TQ               # 8 tq blocks
CK = 128                   # tk chunk (partition dim of S^T tiles)
KD = D // 128              # 8 contraction chunks over D
NV = T // CK               # 32 tk chunks total
VW = HD + 1                # V tile width incl. ones column = 65

BF16 = mybir.dt.bfloat16
F32 = mybir.dt.float32
NPBF16 = ml_dtypes.bfloat16

_CACHE = {}


def _build():
    nc = bacc.Bacc("TRN2", target_bir_lowering=False, debug=False, num_devices=NCORES)
    xT = nc.dram_tensor("xT", [D, T], BF16, kind="ExternalInput").ap()
    # weights are pre-shuffled on host to the SBUF layout [128, KD*E]
    # (chunk-major per partition) so each load is one contiguous-2KB-rows DMA
    wqT = nc.dram_tensor("wqT", [128, KD * E], BF16, kind="ExternalInput").ap()
    wkT = nc.dram_tensor("wkT", [128, KD * E], BF16, kind="ExternalInput").ap()
    wvT = nc.dram_tensor("wvT", [128, KD * E], BF16, kind="ExternalInput").ap()
    wpT = nc.dram_tensor("wpT", [128, KD * E], BF16, kind="ExternalInput").ap()
    mx = nc.dram_tensor("mx", [128, 1024], BF16, kind="ExternalInput").ap()
    out = nc.dram_tensor("out", [T, E], F32, kind="ExternalOutput").ap()
    sendb = nc.dram_tensor("sendb", [NB, 128, TQ], BF16).ap()
    # One gather buffer per collective group of tq blocks; AllGather
    # concatenates by rank, so the per-core slot covers the whole group.
    CGRP = [(0, 1, 2, 3), (4, 5, 6), (7,)]
    gaths = [
        nc.dram_tensor(
            f"gath{gi}", [NCORES, len(g), 128, TQ], BF16, addr_space="Shared"
        ).ap()
        for gi, g in enumerate(CGRP)
    ]

    with tile.TileContext(nc) as tc, ExitStack() as ctx:
        sing = ctx.enter_context(tc.tile_pool(name="sing", bufs=1))
        pwork = ctx.enter_context(tc.tile_pool(name="pwork", bufs=4))
        ynp = ctx.enter_context(tc.tile_pool(name="ynp", bufs=4))
        ygp = ctx.enter_context(tc.tile_pool(name="ygp", bufs=5))
        osb = ctx.enter_context(tc.tile_pool(name="osb", bufs=2))
        # PSUM (8 banks): psS 3 x [128,1024] f32 (2 banks each) rotating between
        # S^T tiles and the filler groups (QKV psum, norm bcast, proj); psY
        # 2 x [65,512] (1 bank each) for the per-block PV accumulators.
        psS = ctx.enter_context(tc.tile_pool(name="psS", bufs=3, space="PSUM"))
        psY = ctx.enter_context(tc.tile_pool(name="psY", bufs=2, space="PSUM"))

        # ---- resident SBUF tensors -------------------------------------
        xT_sb = sing.tile([128, KD * T], BF16)      # d-chunk kc at cols [kc*T, (kc+1)*T)
        wq_sb = sing.tile([128, KD * E], BF16)
        wk_sb = sing.tile([128, KD * E], BF16)
        wv_sb = sing.tile([128, KD * E], BF16)
        wp_sb = sing.tile([128, KD * E], BF16)      # wpT rows chunked, my 128 out cols
        mx_sb = sing.tile([128, 1024], BF16)        # mx[m, j] = 1{j >= m + 512}
        qT_sb = sing.tile([128, T], BF16)           # rows 0:64 head0, 64:128 head1
        kT_sb = sing.tile([128, T], BF16)
        v0_sb = sing.tile([128, NV * VW], BF16)     # V head0 + ones col per chunk
        v1_sb = sing.tile([128, NV * VW], BF16)
        y_sb = sing.tile([128, T], F32)             # unnormalized y^T (both heads)
        dsp_sb = sing.tile([128, 4 * TQ], BF16)     # den rows at partitions {0,32,64,96}
        onesp_sb = sing.tile([128, 128], BF16)
        zb_sb = sing.tile([128, 1], F32)            # zero bias for activations

        nc.vector.memset(zb_sb[:], 0.0)
        nc.vector.memset(onesp_sb[:], 1.0)
        nc.vector.memset(
            v0_sb[:].rearrange("p (c w) -> p c w", w=VW)[:, :, HD : HD + 1], 1.0
        )
        nc.vector.memset(
            v1_sb[:].rearrange("p (c w) -> p c w", w=VW)[:, :, HD : HD + 1], 1.0
        )

        # ---- input DMAs ------------------------------------------------
        # Descriptor generation costs ~550ns + ~0.5ns/descriptor serially on
        # the issuing sequencer, so batch into few large-descriptor DMAs,
        # split across the two HWDGE engines, with the first-wave pieces
        # (wq + x cols [0:1024]) emitted first.
        def load_x(eng, kc, c0, c1):
            eng.dma_start(
                out=xT_sb[:, kc * T + c0 : kc * T + c1],
                in_=xT[kc * CK : (kc + 1) * CK, c0:c1],
            )

        nc.sync.dma_start(out=wq_sb[:], in_=wqT)
        nc.sync.dma_start(out=wk_sb[:], in_=wkT)
        nc.scalar.dma_start(out=mx_sb[:], in_=mx)
        nc.scalar.dma_start(out=wv_sb[:], in_=wvT)
        for kc in range(0, KD, 2):
            load_x(nc.sync, kc, 0, TQ)
            load_x(nc.scalar, kc + 1, 0, TQ)
        for kc in range(0, KD, 2):
            load_x(nc.sync, kc, TQ, 2 * TQ)
            load_x(nc.scalar, kc + 1, TQ, 2 * TQ)
        for kc in range(0, KD, 2):
            load_x(nc.sync, kc, 2 * TQ, 5 * TQ)
            load_x(nc.scalar, kc + 1, 2 * TQ, 5 * TQ)
        nc.scalar.dma_start(out=wp_sb[:], in_=wpT)
        for kc in range(0, KD, 2):
            load_x(nc.sync, kc, 5 * TQ, T)
            load_x(nc.scalar, kc + 1, 5 * TQ, T)



        # ---- QKV helpers (emitted lazily as PE filler) -------------------
        def emit_q(tb):
            ts = tb * TQ
            psq = psS.tile([128, TQ], F32, tag="ps", name=f"psq{tb}")
            for kc in range(KD):
                nc.tensor.matmul(
                    out=psq[:],
                    lhsT=wq_sb[:, kc * E : (kc + 1) * E],
                    rhs=xT_sb[:, kc * T + ts : kc * T + ts + TQ],
                    start=(kc == 0),
                    stop=(kc == KD - 1),
                )
            nc.vector.tensor_copy(out=qT_sb[:, ts : ts + TQ], in_=psq[:])

        def emit_k(tb):
            ts = tb * TQ
            psk = psS.tile([128, TQ], F32, tag="ps", name=f"psk{tb}")
            for kc in range(KD):
                nc.tensor.matmul(
                    out=psk[:],
                    lhsT=wk_sb[:, kc * E : (kc + 1) * E],
                    rhs=xT_sb[:, kc * T + ts : kc * T + ts + TQ],
                    start=(kc == 0),
                    stop=(kc == KD - 1),
                )
            nc.vector.tensor_copy(out=kT_sb[:, ts : ts + TQ], in_=psk[:])

        def emit_v(ci):
            psv = psS.tile([128, E], F32, tag="ps", name=f"psv{ci}")
            for kc in range(KD):
                nc.tensor.matmul(
                    out=psv[:],
                    lhsT=xT_sb[:, kc * T + ci * CK : kc * T + (ci + 1) * CK],
                    rhs=wv_sb[:, kc * E : (kc + 1) * E],
                    start=(kc == 0),
                    stop=(kc == KD - 1),
                )
            nc.vector.tensor_copy(
                out=v0_sb[:, ci * VW : ci * VW + HD], in_=psv[:, 0:HD]
            )
            nc.vector.tensor_copy(
                out=v1_sb[:, ci * VW : ci * VW + HD], in_=psv[:, HD:E]
            )

        # ---- attention helpers ------------------------------------------
        def emit_qk(b, ci):
            """S^T tile for chunk ci: cols 0:512 head0, 512:1024 head1."""
            ts = b * TQ
            off = ci * CK - ts
            lo = max(off, 0)
            s = psS.tile([128, 2 * TQ], F32, tag="ps", name=f"s{b}_{ci}")
            for h in range(2):
                nc.tensor.matmul(
                    out=s[:, h * TQ + lo : (h + 1) * TQ],
                    lhsT=kT_sb[h * HD : (h + 1) * HD, ci * CK : (ci + 1) * CK],
                    rhs=qT_sb[h * HD : (h + 1) * HD, ts + lo : ts + TQ],
                    start=True,
                    stop=True,
                )
            return s

        def emit_exp(b, ci, s):
            ts = b * TQ
            off = ci * CK - ts
            p = pwork.tile([128, 2 * TQ], BF16, tag="pt", name=f"p{b}_{ci}")
            if off <= 0:
                nc.scalar.activation(
                    out=p[:], in_=s[:], func=mybir.ActivationFunctionType.Exp,
                    bias=zb_sb[:],
                )
            else:
                # only the causally reachable columns [off:512] of each head
                pr = p[:].rearrange("p (two t) -> p two t", two=2)[:, :, off:TQ]
                sr = s[:].rearrange("p (two t) -> p two t", two=2)[:, :, off:TQ]
                nc.scalar.activation(
                    out=pr, in_=sr, func=mybir.ActivationFunctionType.Exp,
                    bias=zb_sb[:],
                )
            if off >= 0:
                # zero the masked region (incl. stale cols < off):
                # mx[m, n + 512 - off] = 1{n >= m + off}
                for h in range(2):
                    nc.vector.tensor_mul(
                        p[:, h * TQ : (h + 1) * TQ],
                        p[:, h * TQ : (h + 1) * TQ],
                        mx_sb[:, TQ - off : 2 * TQ - off],
                    )
            return p

        def emit_pv(b, ci, p, y0, y1):
            nch = 4 * (b + 1)
            nc.tensor.matmul(
                out=y0[:],
                lhsT=v0_sb[:, ci * VW : (ci + 1) * VW],
                rhs=p[:, 0:TQ],
                start=(ci == 0),
                stop=(ci == nch - 1),
            )
            nc.tensor.matmul(
                out=y1[:],
                lhsT=v1_sb[:, ci * VW : (ci + 1) * VW],
                rhs=p[:, TQ : 2 * TQ],
                start=(ci == 0),
                stop=(ci == nch - 1),
            )

        # p tiles are only partially written on diagonal chunks (exp skips
        # cols the mask zeroes); pre-zero the 3 rotating buffers so stale
        # regions are always finite (NaN * 0 = NaN otherwise).
        for i in range(4):
            pini = pwork.tile([128, 2 * TQ], BF16, tag="pt", name=f"pini{i}")
            nc.vector.memset(pini[:], 0.0)

        ygtiles = {}

        def emit_norm(b):
            # K=1 bf16 matmuls broadcast the denominator rows across
            # partitions, then 1/x and one multiply; ship block to DRAM.
            rbb = psS.tile([128, TQ], F32, tag="ps", name=f"rbb{b}")
            for h in range(2):
                i = 2 * b + h
                pr = (i % 4) * 32
                cr = (i // 4) * TQ
                nc.tensor.matmul(
                    out=rbb[h * HD : (h + 1) * HD, :],
                    lhsT=onesp_sb[pr : pr + 1, 0:HD],
                    rhs=dsp_sb[pr : pr + 1, cr : cr + TQ],
                    start=True,
                    stop=True,
                    tile_position=(pr, h * HD),
                )
            rq = ynp.tile([128, TQ], F32, tag="rq", name=f"rq{b}")
            nc.vector.reciprocal_approx_fast(out=rq[:], in_=rbb[:])
            yn = ynp.tile([128, TQ], BF16, tag="yn", name=f"yn{b}")
            nc.vector.tensor_mul(yn[:], y_sb[:, b * TQ : (b + 1) * TQ], rq[:])
            nc.sync.dma_start(out=sendb[b], in_=yn[:])

        def emit_gather(gi):
            grp = CGRP[gi]
            b0 = grp[0]
            nc.gpsimd.collective_compute(
                "AllGather",
                mybir.AluOpType.bypass,
                replica_groups=[list(range(NCORES))],
                ins=[sendb[b0 : b0 + len(grp)].opt()],
                outs=[gaths[gi].opt()],
            )

        def emit_recv(gi, tail=False, only=None):
            grp = CGRP[gi]
            nsplit = 4 if tail else 2
            for bi, b in enumerate(grp):
                if only is not None and b not in only:
                    continue
                yg = ygp.tile([128, KD * TQ], BF16, tag="yg", name=f"yg{b}")
                for jh in range(nsplit):
                    j0 = jh * (NCORES // nsplit)
                    nc.gpsimd.dma_start(
                        out=yg[:].rearrange("p (j t) -> p j t", t=TQ)[
                            :, j0 : j0 + NCORES // nsplit
                        ],
                        in_=gaths[gi][j0 : j0 + NCORES // nsplit, bi].rearrange(
                            "j p t -> p j t"
                        ),
                    )
                ygtiles[b] = yg

        def emit_proj(b):
            yg = ygtiles.pop(b)
            po = psS.tile([128, TQ], F32, tag="ps", name=f"po{b}")
            for mt in range(4):
                for j in range(KD):
                    nc.tensor.matmul(
                        out=po[:, mt * 128 : (mt + 1) * 128],
                        lhsT=yg[:, j * TQ + mt * 128 : j * TQ + (mt + 1) * 128],
                        rhs=wp_sb[:, j * E : (j + 1) * E],
                        start=(j == 0),
                        stop=(j == KD - 1),
                    )
            ob = osb.tile([128, TQ], F32, tag="o", name=f"ob{b}")
            nc.vector.tensor_copy(out=ob[:], in_=po[:])
            for mh in range(2):
                nc.sync.dma_start(
                    out=out[b * TQ + mh * 256 : b * TQ + (mh + 1) * 256, :].rearrange(
                        "(mt p) e -> p mt e", p=128
                    ),
                    in_=ob[:].rearrange("p (mt e) -> p mt e", mt=4)[
                        :, mh * 2 : (mh + 1) * 2
                    ],
                )

        def emit_stash(b, y0, y1):
            ts = b * TQ
            for h, yy in ((0, y0), (1, y1)):
                i = 2 * b + h
                nc.vector.tensor_copy(
                    out=dsp_sb[
                        (i % 4) * 32 : (i % 4) * 32 + 1,
                        (i // 4) * TQ : (i // 4 + 1) * TQ,
                    ],
                    in_=yy[HD : HD + 1, :],
                )
            nc.vector.tensor_copy(out=y_sb[0:HD, ts : ts + TQ], in_=y0[0:HD, :])
            nc.vector.tensor_copy(out=y_sb[HD:128, ts : ts + TQ], in_=y1[0:HD, :])

        # ---- main loop ---------------------------------------------------
        emit_q(0)
        emit_k(0)
        for ci in range(4):
            emit_v(ci)

        for b in range(NB):
            nch = 4 * (b + 1)
            fillers = []
            if b >= 1:
                fillers.append(lambda bb=b - 1: emit_norm(bb))
            if b == 4:
                fillers.append(lambda: emit_gather(0))
            if b == 5:
                fillers.append(lambda: emit_recv(0))
            if b == 7:
                fillers.append(lambda: emit_gather(1))
                fillers.append(lambda: emit_recv(1, only=(4, 5)))
            if b + 1 < NB:
                fillers.append(lambda tb=b + 1: emit_q(tb))
                fillers.append(lambda tb=b + 1: emit_k(tb))
                fillers += [lambda c=ci: emit_v(c) for ci in range(4 * b + 4, 4 * b + 8)]
            if b == 6:
                fillers.append(lambda: emit_proj(0))
            if b == 7:
                fillers += [lambda bb=bb: emit_proj(bb) for bb in (1, 2, 3, 4, 5)]

            y0 = psY.tile([VW, TQ], F32, tag="py", name=f"y0_{b}")
            y1 = psY.tile([VW, TQ], F32, tag="py", name=f"y1_{b}")
            s = {0: emit_qk(b, 0)}
            if nch > 1:
                s[1] = emit_qk(b, 1)
            p = {0: emit_exp(b, 0, s.pop(0))}
            nf = 0
            for ci in range(nch):
                if ci + 2 < nch:
                    s[ci + 2] = emit_qk(b, ci + 2)
                if ci + 1 < nch:
                    p[ci + 1] = emit_exp(b, ci + 1, s.pop(ci + 1))
                # fillers between QK and PV: if PV's dep is late, the PE
                # head-of-line still has ready work
                want = ((ci + 1) * len(fillers)) // nch
                while nf < want:
                    fillers[nf]()
                    nf += 1
                emit_pv(b, ci, p.pop(ci), y0, y1)
            while nf < len(fillers):
                fillers[nf]()
                nf += 1
            emit_stash(b, y0, y1)

        emit_norm(NB - 1)
        emit_gather(2)
        emit_recv(1, only=(6,))
        emit_proj(NB - 2)
        emit_recv(2, tail=True)
        emit_proj(NB - 1)

    nc.compile()
    return nc


def _inputs(x, w_attn, w_proj):
    x = np.asarray(x, dtype=np.float32).reshape(T, D)
    w_attn = np.asarray(w_attn, dtype=np.float32)
    w_proj = np.asarray(w_proj, dtype=np.float32)

    xT_np = np.ascontiguousarray(x.T).astype(NPBF16)
    wpT_np = np.ascontiguousarray(w_proj.T)
    scale = 1.0 / math.sqrt(HD)
    mx_np = (
        np.arange(1024, dtype=np.int32)[None, :]
        >= (np.arange(128, dtype=np.int32)[:, None] + 512)
    ).astype(NPBF16)

    def shuf(wT):  # [D, E] -> SBUF layout [128, KD*E], 2KB-contiguous rows
        return np.ascontiguousarray(
            wT.reshape(KD, 128, E).transpose(1, 0, 2).reshape(128, KD * E)
        ).astype(NPBF16)

    in_maps = []
    for core in range(NCORES):
        r0 = core * E
        in_maps.append(
            {
                "xT": xT_np,
                "wqT": shuf((w_attn[r0 : r0 + E, :] * scale).T),
                "wkT": shuf(w_attn[D + r0 : D + r0 + E, :].T),
                "wvT": shuf(w_attn[2 * D + r0 : 2 * D + r0 + E, :].T),
                "wpT": shuf(wpT_np[:, r0 : r0 + E]),
                "mx": mx_np,
            }
        )
    return in_maps


def kernel(x, w_attn, w_proj, _trace=False):
    if "nc" not in _CACHE:
        _CACHE["nc"] = _build()
    nc = _CACHE["nc"]
    in_maps = _inputs(x, w_attn, w_proj)
    res = run_bass_kernel_spmd(
        nc, in_maps, core_ids=list(range(NCORES)), trace=_trace
    )
    _CACHE["last_result"] = res
    full = np.concatenate([res.results[c]["out"] for c in range(NCORES)], axis=1)
    return full.reshape(B, T, D).astype(np.float32)


# revision 40
# speedup vs baseline: 1.0691x; 1.0691x over previous
"""Causal self-attention (B=1, T=4096, D=1024, H=16, HD=64) on 8 trn2 NeuronCores.

Sharding: tensor-parallel over heads (2 heads per core) for QKV + attention.
The output projection is COLUMN-sharded (core c computes out[:, c*128:(c+1)*128])
so the re-shard collective can be split into one small AllGather per tq block,
each fully overlapped with the remaining attention compute; the projection for
block b runs as PE filler during block b+2.  Only the last block's gather +
projection (~2% of work) sits on the critical-path tail, vs. a monolithic
end-of-kernel AllToAll + projection in the row-sharded layout.

Matmul layout notes (PE computes out = lhsT.T @ rhs, contraction on partitions):
 - host feeds x transposed (xT [D, T]) so QKV needs no on-chip transposes.
 - S^T tiles [tk, tq] are computed (not S) so the PV matmul can consume
   exp(S^T) directly as the moving operand with V in natural [tk, hd] layout.
 - a ones-column appended to V makes row 64 of the PV accumulator the
   softmax denominator (no extra reduction pass).
 - softmax max-subtraction is skipped: scores are ~N(0,1) (|s| < ~10), and
   a constant shift cancels exactly in softmax, so exp is safe in fp32.
 - causal masking via a DVE multiply with a sliced triangular bf16 mask
   (cheaper than burning PE columns accumulating -1e9); diagonal-chunk QK
   matmuls and exps only cover the causally-reachable column range.
 - attention inner loop is software-pipelined one chunk per iteration
   (QK(ci+2) | exp(ci+1) | PV(ci)) so the PE never waits on the Scalar
   engine's exp chain; QKV projections for the next block, normalization of
   the previous block, and the output projection are interleaved as PE filler.
"""

import math
import sys
from contextlib import ExitStack

sys.path.insert(0, "/opt/trn_rl_repo")

import ml_dtypes
import numpy as np

import concourse.bass as bass  # noqa: F401  (bass types used via tile/bacc)
import concourse.mybir as mybir
import concourse.tile as tile
from concourse import bacc
from concourse.bass_utils import run_bass_kernel_spmd

B, T, D, H, HD = 1, 4096, 1024, 16, 64
NCORES = 8
HPC = H // NCORES          # heads per core = 2
E = HPC * HD               # per-core head width = 128
TQ = 512                   # tq block width
NB = T // TQ               # 8 tq blocks
CK = 128                   # tk chunk (partition dim of S^T tiles)
KD = D // 128              # 8 contraction chunks over D
NV = T // CK               # 32 tk chunks total
VW = HD + 1                # V tile width incl. ones column = 65

BF16 = mybir.dt.bfloat16
F32 = mybir.dt.float32
NPBF16 = ml_dtypes.bfloat16

_CACHE = {}


def _build():
    nc = bacc.Bacc("TRN2", target_bir_lowering=False, debug=False, num_devices=NCORES)
    xT = nc.dram_tensor("xT", [D, T], BF16, kind="ExternalInput").ap()
    # weights are pre-shuffled on host to the SBUF layout [128, KD*E]
    # (chunk-major per partition) so each load is one contiguous-2KB-rows DMA
    wqT = nc.dram_tensor("wqT", [128, KD * E], BF16, kind="ExternalInput").ap()
    wkT = nc.dram_tensor("wkT", [128, KD * E], BF16, kind="ExternalInput").ap()
    wvT = nc.dram_tensor("wvT", [128, KD * E], BF16, kind="ExternalInput").ap()
    wpT = nc.dram_tensor("wpT", [128, KD * E], BF16, kind="ExternalInput").ap()
    mx = nc.dram_tensor("mx", [128, 1024], BF16, kind="ExternalInput").ap()
    out = nc.dram_tensor("out", [T, E], F32, kind="ExternalOutput").ap()
    sendb = nc.dram_tensor("sendb", [NB, 128, TQ], BF16).ap()
    # One gather buffer per collective group of tq blocks; AllGather
    # concatenates by rank, so the per-core slot covers the whole group.
    CGRP = [(0, 1, 2, 3), (4, 5, 6), (7,)]
    gaths = [
        nc.dram_tensor(
            f"gath{gi}", [NCORES, len(g), 128, TQ], BF16, addr_space="Shared"
        ).ap()
        for gi, g in enumerate(CGRP)
    ]

    with tile.TileContext(nc) as tc, ExitStack() as ctx:
        sing = ctx.enter_context(tc.tile_pool(name="sing", bufs=1))
        pwork = ctx.enter_context(tc.tile_pool(name="pwork", bufs=4))
        ynp = ctx.enter_context(tc.tile_pool(name="ynp", bufs=4))
        ygp = ctx.enter_context(tc.tile_pool(name="ygp", bufs=5))
        osb = ctx.enter_context(tc.tile_pool(name="osb", bufs=2))
        # PSUM (8 banks): psS 3 x [128,1024] f32 (2 banks each) rotating between
        # S^T tiles and the filler groups (QKV psum, norm bcast, proj); psY
        # 2 x [65,512] (1 bank each) for the per-block PV accumulators.
        psS = ctx.enter_context(tc.tile_pool(name="psS", bufs=3, space="PSUM"))
        psY = ctx.enter_context(tc.tile_pool(name="psY", bufs=2, space="PSUM"))

        # ---- resident SBUF tensors -------------------------------------
        xT_sb = sing.tile([128, KD * T], BF16)      # d-chunk kc at cols [kc*T, (kc+1)*T)
        wq_sb = sing.tile([128, KD * E], BF16)
        wk_sb = sing.tile([128, KD * E], BF16)
        wv_sb = sing.tile([128, KD * E], BF16)
        wp_sb = sing.tile([128, KD * E], BF16)      # wpT rows chunked, my 128 out cols
        mx_sb = sing.tile([128, 1024], BF16)        # mx[m, j] = 1{j >= m + 512}
        qT_sb = sing.tile([128, T], BF16)           # rows 0:64 head0, 64:128 head1
        kT_sb = sing.tile([128, T], BF16)
        v0_sb = sing.tile([128, NV * VW], BF16)     # V head0 + ones col per chunk
        v1_sb = sing.tile([128, NV * VW], BF16)
        y_sb = sing.tile([128, T], F32)             # unnormalized y^T (both heads)
        dsp_sb = sing.tile([128, 4 * TQ], BF16)     # den rows at partitions {0,32,64,96}
        onesp_sb = sing.tile([128, 128], BF16)
        zb_sb = sing.tile([128, 1], F32)            # zero bias for activations

        nc.vector.memset(zb_sb[:], 0.0)
        nc.vector.memset(onesp_sb[:], 1.0)
        nc.vector.memset(
            v0_sb[:].rearrange("p (c w) -> p c w", w=VW)[:, :, HD : HD + 1], 1.0
        )
        nc.vector.memset(
            v1_sb[:].rearrange("p (c w) -> p c w", w=VW)[:, :, HD : HD + 1], 1.0
        )

        # ---- input DMAs ------------------------------------------------
        # Descriptor generation costs ~550ns + ~0.5ns/descriptor serially on
        # the issuing sequencer, so batch into few large-descriptor DMAs,
        # split across the two HWDGE engines, with the first-wave pieces
        # (wq + x cols [0:1024]) emitted first.
        def load_x(eng, kc, c0, c1):
            eng.dma_start(
                out=xT_sb[:, kc * T + c0 : kc * T + c1],
                in_=xT[kc * CK : (kc + 1) * CK, c0:c1],
            )

        nc.sync.dma_start(out=wq_sb[:], in_=wqT)
        nc.sync.dma_start(out=wk_sb[:], in_=wkT)
        nc.scalar.dma_start(out=mx_sb[:], in_=mx)
        nc.scalar.dma_start(out=wv_sb[:], in_=wvT)
        for kc in range(0, KD, 2):
            load_x(nc.sync, kc, 0, TQ)
            load_x(nc.scalar, kc + 1, 0, TQ)
        for kc in range(0, KD, 2):
            load_x(nc.sync, kc, TQ, 2 * TQ)
            load_x(nc.scalar, kc + 1, TQ, 2 * TQ)
        for kc in range(0, KD, 2):
            load_x(nc.sync, kc, 2 * TQ, 5 * TQ)
            load_x(nc.scalar, kc + 1, 2 * TQ, 5 * TQ)
        nc.scalar.dma_start(out=wp_sb[:], in_=wpT)
        for kc in range(0, KD, 2):
            load_x(nc.sync, kc, 5 * TQ, T)
            load_x(nc.scalar, kc + 1, 5 * TQ, T)



        # ---- QKV helpers (emitted lazily as PE filler) -------------------
        def emit_q(tb):
            ts = tb * TQ
            psq = psS.tile([128, TQ], F32, tag="ps", name=f"psq{tb}")
            for kc in range(KD):
                nc.tensor.matmul(
                    out=psq[:],
                    lhsT=wq_sb[:, kc * E : (kc + 1) * E],
                    rhs=xT_sb[:, kc * T + ts : kc * T + ts + TQ],
                    start=(kc == 0),
                    stop=(kc == KD - 1),
                )
            nc.vector.tensor_copy(out=qT_sb[:, ts : ts + TQ], in_=psq[:])

        def emit_k(tb):
            ts = tb * TQ
            psk = psS.tile([128, TQ], F32, tag="ps", name=f"psk{tb}")
            for kc in range(KD):
                nc.tensor.matmul(
                    out=psk[:],
                    lhsT=wk_sb[:, kc * E : (kc + 1) * E],
                    rhs=xT_sb[:, kc * T + ts : kc * T + ts + TQ],
                    start=(kc == 0),
                    stop=(kc == KD - 1),
                )
            nc.vector.tensor_copy(out=kT_sb[:, ts : ts + TQ], in_=psk[:])

        def emit_v(ci):
            psv = psS.tile([128, E], F32, tag="ps", name=f"psv{ci}")
            for kc in range(KD):
                nc.tensor.matmul(
                    out=psv[:],
                    lhsT=xT_sb[:, kc * T + ci * CK : kc * T + (ci + 1) * CK],
                    rhs=wv_sb[:, kc * E : (kc + 1) * E],
                    start=(kc == 0),
                    stop=(kc == KD - 1),
                )
            nc.vector.tensor_copy(
                out=v0_sb[:, ci * VW : ci * VW + HD], in_=psv[:, 0:HD]
            )
            nc.vector.tensor_copy(
                out=v1_sb[:, ci * VW : ci * VW + HD], in_=psv[:, HD:E]
            )

        # ---- attention helpers ------------------------------------------
        def emit_qk(b, ci):
            """S^T tile for chunk ci: cols 0:512 head0, 512:1024 head1."""
            ts = b * TQ
            off = ci * CK - ts
            lo = max(off, 0)
            s = psS.tile([128, 2 * TQ], F32, tag="ps", name=f"s{b}_{ci}")
            for h in range(2):
                nc.tensor.matmul(
                    out=s[:, h * TQ + lo : (h + 1) * TQ],
                    lhsT=kT_sb[h * HD : (h + 1) * HD, ci * CK : (ci + 1) * CK],
                    rhs=qT_sb[h * HD : (h + 1) * HD, ts + lo : ts + TQ],
                    start=True,
                    stop=True,
                )
            return s

        def emit_exp(b, ci, s):
            ts = b * TQ
            off = ci * CK - ts
            p = pwork.tile([128, 2 * TQ], BF16, tag="pt", name=f"p{b}_{ci}")
            if off <= 0:
                nc.scalar.activation(
                    out=p[:], in_=s[:], func=mybir.ActivationFunctionType.Exp,
                    bias=zb_sb[:],
                )
            else:
                # only the causally reachable columns [off:512] of each head
                pr = p[:].rearrange("p (two t) -> p two t", two=2)[:, :, off:TQ]
                sr = s[:].rearrange("p (two t) -> p two t", two=2)[:, :, off:TQ]
                nc.scalar.activation(
                    out=pr, in_=sr, func=mybir.ActivationFunctionType.Exp,
                    bias=zb_sb[:],
                )
            if off >= 0:
                # zero the masked region (incl. stale cols < off):
                # mx[m, n + 512 - off] = 1{n >= m + off}
                for h in range(2):
                    nc.vector.tensor_mul(
                        p[:, h * TQ : (h + 1) * TQ],
                        p[:, h * TQ : (h + 1) * TQ],
                        mx_sb[:, TQ - off : 2 * TQ - off],
                    )
            return p

        def emit_pv(b, ci, p, y0, y1):
            nch = 4 * (b + 1)
            nc.tensor.matmul(
                out=y0[:],
                lhsT=v0_sb[:, ci * VW : (ci + 1) * VW],
                rhs=p[:, 0:TQ],
                start=(ci == 0),
                stop=(ci == nch - 1),
            )
            nc.tensor.matmul(
                out=y1[:],
                lhsT=v1_sb[:, ci * VW : (ci + 1) * VW],
                rhs=p[:, TQ : 2 * TQ],
                start=(ci == 0),
                stop=(ci == nch - 1),
            )

        # p tiles are only partially written on diagonal chunks (exp skips
        # cols the mask zeroes); pre-zero the 3 rotating buffers so stale
        # regions are always finite (NaN * 0 = NaN otherwise).
        for i in range(4):
            pini = pwork.tile([128, 2 * TQ], BF16, tag="pt", name=f"pini{i}")
            nc.vector.memset(pini[:], 0.0)

        ygtiles = {}

        def emit_norm(b):
            # K=1 bf16 matmuls broadcast the denominator rows across
            # partitions, then 1/x and one multiply; ship block to DRAM.
            rbb = psS.tile([128, TQ], F32, tag="ps", name=f"rbb{b}")
            for h in range(2):
                i = 2 * b + h
                pr = (i % 4) * 32
                cr = (i // 4) * TQ
                nc.tensor.matmul(
                    out=rbb[h * HD : (h + 1) * HD, :],
                    lhsT=onesp_sb[pr : pr + 1, 0:HD],
                    rhs=dsp_sb[pr : pr + 1, cr : cr + TQ],
                    start=True,
                    stop=True,
                    tile_position=(pr, h * HD),
                )
            rq = ynp.tile([128, TQ], F32, tag="rq", name=f"rq{b}")
            nc.vector.reciprocal_approx_fast(out=rq[:], in_=rbb[:])
            yn = ynp.tile([128, TQ], BF16, tag="yn", name=f"yn{b}")
            nc.vector.tensor_mul(yn[:], y_sb[:, b * TQ : (b + 1) * TQ], rq[:])
            nc.sync.dma_start(out=sendb[b], in_=yn[:])

        def emit_gather(gi):
            grp = CGRP[gi]
            b0 = grp[0]
            nc.gpsimd.collective_compute(
                "AllGather",
                mybir.AluOpType.bypass,
                replica_groups=[list(range(NCORES))],
                ins=[sendb[b0 : b0 + len(grp)].opt()],
                outs=[gaths[gi].opt()],
            )

        def emit_recv(gi, tail=False, only=None):
            grp = CGRP[gi]
            nsplit = 4 if tail else 2
            for bi, b in enumerate(grp):
                if only is not None and b not in only:
                    continue
                yg = ygp.tile([128, KD * TQ], BF16, tag="yg", name=f"yg{b}")
                for jh in range(nsplit):
                    j0 = jh * (NCORES // nsplit)
                    nc.gpsimd.dma_start(
                        out=yg[:].rearrange("p (j t) -> p j t", t=TQ)[
                            :, j0 : j0 + NCORES // nsplit
                        ],
                        in_=gaths[gi][j0 : j0 + NCORES // nsplit, bi].rearrange(
                            "j p t -> p j t"
                        ),
                    )
                ygtiles[b] = yg

        def emit_proj(b):
            yg = ygtiles.pop(b)
            po = psS.tile([128, TQ], F32, tag="ps", name=f"po{b}")
            for mt in range(4):
                for j in range(KD):
                    nc.tensor.matmul(
                        out=po[:, mt * 128 : (mt + 1) * 128],
                        lhsT=yg[:, j * TQ + mt * 128 : j * TQ + (mt + 1) * 128],
                        rhs=wp_sb[:, j * E : (j + 1) * E],
                        start=(j == 0),
                        stop=(j == KD - 1),
                    )
            ob = osb.tile([128, TQ], F32, tag="o", name=f"ob{b}")
            nc.vector.tensor_copy(out=ob[:], in_=po[:])
            for mh in range(2):
                nc.sync.dma_start(
                    out=out[b * TQ + mh * 256 : b * TQ + (mh + 1) * 256, :].rearrange(
                        "(mt p) e -> p mt e", p=128
                    ),
                    in_=ob[:].rearrange("p (mt e) -> p mt e", mt=4)[
                        :, mh * 2 : (mh + 1) * 2
                    ],
                )

        def emit_stash(b, y0, y1):
            ts = b * TQ
            for h, yy in ((0, y0), (1, y1)):
                i = 2 * b + h
                nc.vector.tensor_copy(
                    out=dsp_sb[
                        (i % 4) * 32 : (i % 4) * 32 + 1,
                        (i // 4) * TQ : (i // 4 + 1) * TQ,
                    ],
                    in_=yy[HD : HD + 1, :],
                )
            nc.vector.tensor_copy(out=y_sb[0:HD, ts : ts + TQ], in_=y0[0:HD, :])
            nc.vector.tensor_copy(out=y_sb[HD:128, ts : ts + TQ], in_=y1[0:HD, :])

        # ---- main loop ---------------------------------------------------
        emit_q(0)
        emit_k(0)
        for ci in range(4):
            emit_v(ci)

        for b in range(NB):
            nch = 4 * (b + 1)
            fillers = []
            if b >= 1:
                fillers.append(lambda bb=b - 1: emit_norm(bb))
            if b == 4:
                fillers.append(lambda: emit_gather(0))
            if b == 5:
                fillers.append(lambda: emit_recv(0))
            if b == 7:
                fillers.append(lambda: emit_gather(1))
            if b + 1 < NB:
                fillers.append(lambda tb=b + 1: emit_q(tb))
                fillers.append(lambda tb=b + 1: emit_k(tb))
                fillers += [lambda c=ci: emit_v(c) for ci in range(4 * b + 4, 4 * b + 8)]
            if b == 6:
                fillers.append(lambda: emit_proj(0))
            if b == 7:
                # only AG0-backed projections (data delivered blocks ago);
                # blocks 4-6 project in the epilogue under the tail mesh wait
                fillers += [lambda bb=bb: emit_proj(bb) for bb in (1, 2, 3)]

            y0 = psY.tile([VW, TQ], F32, tag="py", name=f"y0_{b}")
            y1 = psY.tile([VW, TQ], F32, tag="py", name=f"y1_{b}")
            s = {0: emit_qk(b, 0)}
            if nch > 1:
                s[1] = emit_qk(b, 1)
            p = {0: emit_exp(b, 0, s.pop(0))}
            nf = 0
            for ci in range(nch):
                if ci + 2 < nch:
                    s[ci + 2] = emit_qk(b, ci + 2)
                if ci + 1 < nch:
                    p[ci + 1] = emit_exp(b, ci + 1, s.pop(ci + 1))
                # fillers between QK and PV: if PV's dep is late, the PE
                # head-of-line still has ready work; drain one iter early so
                # the block transition isn't a filler dump
                want = ((ci + 1) * len(fillers)) // max(nch - 1, 1)
                while nf < min(want, len(fillers)):
                    fillers[nf]()
                    nf += 1
                emit_pv(b, ci, p.pop(ci), y0, y1)
            emit_stash(b, y0, y1)
            while nf < len(fillers):
                fillers[nf]()
                nf += 1

        emit_norm(NB - 1)
        emit_gather(2)
        emit_recv(1)
        emit_proj(4)
        emit_proj(5)
        emit_proj(6)
        emit_recv(2, tail=True)
        emit_proj(NB - 1)

    nc.compile()
    return nc


def _inputs(x, w_attn, w_proj):
    x = np.asarray(x, dtype=np.float32).reshape(T, D)
    w_attn = np.asarray(w_attn, dtype=np.float32)
    w_proj = np.asarray(w_proj, dtype=np.float32)

    xT_np = np.ascontiguousarray(x.T).astype(NPBF16)
    wpT_np = np.ascontiguousarray(w_proj.T)
    scale = 1.0 / math.sqrt(HD)
    mx_np = (
        np.arange(1024, dtype=np.int32)[None, :]
        >= (np.arange(128, dtype=np.int32)[:, None] + 512)
    ).astype(NPBF16)

    def shuf(wT):  # [D, E] -> SBUF layout [128, KD*E], 2KB-contiguous rows
        return np.ascontiguousarray(
            wT.reshape(KD, 128, E).transpose(1, 0, 2).reshape(128, KD * E)
        ).astype(NPBF16)

    in_maps = []
    for core in range(NCORES):
        r0 = core * E
        in_maps.append(
            {
                "xT": xT_np,
                "wqT": shuf((w_attn[r0 : r0 + E, :] * scale).T),
                "wkT": shuf(w_attn[D + r0 : D + r0 + E, :].T),
                "wvT": shuf(w_attn[2 * D + r0 : 2 * D + r0 + E, :].T),
                "wpT": shuf(wpT_np[:, r0 : r0 + E]),
                "mx": mx_np,
            }
        )
    return in_maps


def kernel(x, w_attn, w_proj, _trace=False):
    if "nc" not in _CACHE:
        _CACHE["nc"] = _build()
    nc = _CACHE["nc"]
    in_maps = _inputs(x, w_attn, w_proj)
    res = run_bass_kernel_spmd(
        nc, in_maps, core_ids=list(range(NCORES)), trace=_trace
    )
    _CACHE["last_result"] = res
    full = np.concatenate([res.results[c]["out"] for c in range(NCORES)], axis=1)
    return full.reshape(B, T, D).astype(np.float32)


# revision 44
# speedup vs baseline: 1.1345x; 1.0612x over previous
"""Causal self-attention (B=1, T=4096, D=1024, H=16, HD=64) on 8 trn2 NeuronCores.

Sharding: tensor-parallel over heads (2 heads per core) for QKV + attention.
The output projection is COLUMN-sharded (core c computes out[:, c*128:(c+1)*128])
so the re-shard collective can be split into one small AllGather per tq block,
each fully overlapped with the remaining attention compute; the projection for
block b runs as PE filler during block b+2.  Only the last block's gather +
projection (~2% of work) sits on the critical-path tail, vs. a monolithic
end-of-kernel AllToAll + projection in the row-sharded layout.

Matmul layout notes (PE computes out = lhsT.T @ rhs, contraction on partitions):
 - host feeds x transposed (xT [D, T]) so QKV needs no on-chip transposes.
 - S^T tiles [tk, tq] are computed (not S) so the PV matmul can consume
   exp(S^T) directly as the moving operand with V in natural [tk, hd] layout.
 - a ones-column appended to V makes row 64 of the PV accumulator the
   softmax denominator (no extra reduction pass).
 - softmax max-subtraction is skipped: scores are ~N(0,1) (|s| < ~10), and
   a constant shift cancels exactly in softmax, so exp is safe in fp32.
 - causal masking via a DVE multiply with a sliced triangular bf16 mask
   (cheaper than burning PE columns accumulating -1e9); diagonal-chunk QK
   matmuls and exps only cover the causally-reachable column range.
 - attention inner loop is software-pipelined one chunk per iteration
   (QK(ci+2) | exp(ci+1) | PV(ci)) so the PE never waits on the Scalar
   engine's exp chain; QKV projections for the next block, normalization of
   the previous block, and the output projection are interleaved as PE filler.
"""

import math
import sys
from contextlib import ExitStack

sys.path.insert(0, "/opt/trn_rl_repo")

import ml_dtypes
import numpy as np

import concourse.bass as bass  # noqa: F401  (bass types used via tile/bacc)
import concourse.mybir as mybir
import concourse.tile as tile
from concourse import bacc
from concourse.bass_utils import run_bass_kernel_spmd

B, T, D, H, HD = 1, 4096, 1024, 16, 64
NCORES = 8
HPC = H // NCORES          # heads per core = 2
E = HPC * HD               # per-core head width = 128
TQ = 512                   # tq block width
NB = T // TQ               # 8 tq blocks
CK = 128                   # tk chunk (partition dim of S^T tiles)
KD = D // 128              # 8 contraction chunks over D
NV = T // CK               # 32 tk chunks total
VW = HD + 1                # V tile width incl. ones column = 65

BF16 = mybir.dt.bfloat16
F32 = mybir.dt.float32
NPBF16 = ml_dtypes.bfloat16

_CACHE = {}


def _build():
    nc = bacc.Bacc("TRN2", target_bir_lowering=False, debug=False, num_devices=NCORES)
    xT = nc.dram_tensor("xT", [D, T], BF16, kind="ExternalInput").ap()
    # weights are pre-shuffled on host to the SBUF layout [128, KD*E]
    # (chunk-major per partition) so each load is one contiguous-2KB-rows DMA
    wqT = nc.dram_tensor("wqT", [128, KD * E], BF16, kind="ExternalInput").ap()
    wkT = nc.dram_tensor("wkT", [128, KD * E], BF16, kind="ExternalInput").ap()
    wvT = nc.dram_tensor("wvT", [128, KD * E], BF16, kind="ExternalInput").ap()
    wpT = nc.dram_tensor("wpT", [128, KD * E], BF16, kind="ExternalInput").ap()
    mx = nc.dram_tensor("mx", [128, 1024], BF16, kind="ExternalInput").ap()
    out = nc.dram_tensor("out", [T, E], F32, kind="ExternalOutput").ap()
    sendb = nc.dram_tensor("sendb", [NB, 128, TQ], BF16).ap()
    # One gather buffer per collective group of tq blocks; AllGather
    # concatenates by rank, so the per-core slot covers the whole group.
    CGRP = [(0, 1, 2, 3), (4, 5, 6), (7,)]
    gaths = [
        nc.dram_tensor(
            f"gath{gi}", [NCORES, len(g), 128, TQ], BF16, addr_space="Shared"
        ).ap()
        for gi, g in enumerate(CGRP)
    ]

    with tile.TileContext(nc) as tc, ExitStack() as ctx:
        sing = ctx.enter_context(tc.tile_pool(name="sing", bufs=1))
        pwork = ctx.enter_context(tc.tile_pool(name="pwork", bufs=4))
        ynp = ctx.enter_context(tc.tile_pool(name="ynp", bufs=4))
        ygp = ctx.enter_context(tc.tile_pool(name="ygp", bufs=5))
        osb = ctx.enter_context(tc.tile_pool(name="osb", bufs=2))
        # PSUM (8 banks): psS 3 x [128,1024] f32 (2 banks each) rotating between
        # S^T tiles and the filler groups (QKV psum, norm bcast, proj); psY
        # 2 x [65,512] (1 bank each) for the per-block PV accumulators.
        psS = ctx.enter_context(tc.tile_pool(name="psS", bufs=3, space="PSUM"))
        psY = ctx.enter_context(tc.tile_pool(name="psY", bufs=2, space="PSUM"))

        # ---- resident SBUF tensors -------------------------------------
        xT_sb = sing.tile([128, KD * T], BF16)      # d-chunk kc at cols [kc*T, (kc+1)*T)
        wq_sb = sing.tile([128, KD * E], BF16)
        wk_sb = sing.tile([128, KD * E], BF16)
        wv_sb = sing.tile([128, KD * E], BF16)
        wp_sb = sing.tile([128, KD * E], BF16)      # wpT rows chunked, my 128 out cols
        mx_sb = sing.tile([128, 1024], BF16)        # mx[m, j] = 1{j >= m + 512}
        qT_sb = sing.tile([128, T], BF16)           # rows 0:64 head0, 64:128 head1
        kT_sb = sing.tile([128, T], BF16)
        v0_sb = sing.tile([128, NV * VW], BF16)     # V head0 + ones col per chunk
        v1_sb = sing.tile([128, NV * VW], BF16)
        y_sb = sing.tile([128, T], F32)             # unnormalized y^T (both heads)
        dsp_sb = sing.tile([128, 4 * TQ], BF16)     # den rows at partitions {0,32,64,96}
        onesp_sb = sing.tile([128, 128], BF16)
        zb_sb = sing.tile([128, 1], F32)            # zero bias for activations

        nc.vector.memset(zb_sb[:], 0.0)
        nc.vector.memset(onesp_sb[:], 1.0)
        nc.vector.memset(
            v0_sb[:].rearrange("p (c w) -> p c w", w=VW)[:, :, HD : HD + 1], 1.0
        )
        nc.vector.memset(
            v1_sb[:].rearrange("p (c w) -> p c w", w=VW)[:, :, HD : HD + 1], 1.0
        )

        # ---- input DMAs ------------------------------------------------
        # Descriptor generation costs ~550ns + ~0.5ns/descriptor serially on
        # the issuing sequencer, so batch into few large-descriptor DMAs,
        # split across the two HWDGE engines, with the first-wave pieces
        # (wq + x cols [0:1024]) emitted first.
        def load_x(eng, kc, c0, c1):
            eng.dma_start(
                out=xT_sb[:, kc * T + c0 : kc * T + c1],
                in_=xT[kc * CK : (kc + 1) * CK, c0:c1],
            )

        # everything on sync: descgens on the scalar engine would delay the
        # first exps (the ACT queue is in-order)
        nc.sync.dma_start(out=wq_sb[:], in_=wqT)
        for kc in range(KD):
            load_x(nc.sync, kc, 0, TQ)
        nc.sync.dma_start(out=wk_sb[:], in_=wkT)
        nc.sync.dma_start(out=mx_sb[:], in_=mx)
        nc.sync.dma_start(out=wv_sb[:], in_=wvT)
        for kc in range(KD):
            load_x(nc.sync, kc, TQ, 2 * TQ)
        nc.sync.dma_start(out=wp_sb[:], in_=wpT)
        for kc in range(KD):
            load_x(nc.sync, kc, 2 * TQ, 5 * TQ)
        for kc in range(KD):
            load_x(nc.sync, kc, 5 * TQ, T)



        # ---- QKV helpers (emitted lazily as PE filler) -------------------
        def emit_q(tb):
            ts = tb * TQ
            psq = psS.tile([128, TQ], F32, tag="ps", name=f"psq{tb}")
            for kc in range(KD):
                nc.tensor.matmul(
                    out=psq[:],
                    lhsT=wq_sb[:, kc * E : (kc + 1) * E],
                    rhs=xT_sb[:, kc * T + ts : kc * T + ts + TQ],
                    start=(kc == 0),
                    stop=(kc == KD - 1),
                )
            nc.vector.tensor_copy(out=qT_sb[:, ts : ts + TQ], in_=psq[:])

        def emit_k(tb):
            ts = tb * TQ
            psk = psS.tile([128, TQ], F32, tag="ps", name=f"psk{tb}")
            for kc in range(KD):
                nc.tensor.matmul(
                    out=psk[:],
                    lhsT=wk_sb[:, kc * E : (kc + 1) * E],
                    rhs=xT_sb[:, kc * T + ts : kc * T + ts + TQ],
                    start=(kc == 0),
                    stop=(kc == KD - 1),
                )
            nc.vector.tensor_copy(out=kT_sb[:, ts : ts + TQ], in_=psk[:])

        def emit_v(ci):
            psv = psS.tile([128, E], F32, tag="ps", name=f"psv{ci}")
            for kc in range(KD):
                nc.tensor.matmul(
                    out=psv[:],
                    lhsT=xT_sb[:, kc * T + ci * CK : kc * T + (ci + 1) * CK],
                    rhs=wv_sb[:, kc * E : (kc + 1) * E],
                    start=(kc == 0),
                    stop=(kc == KD - 1),
                )
            nc.vector.tensor_copy(
                out=v0_sb[:, ci * VW : ci * VW + HD], in_=psv[:, 0:HD]
            )
            nc.vector.tensor_copy(
                out=v1_sb[:, ci * VW : ci * VW + HD], in_=psv[:, HD:E]
            )

        # ---- attention helpers ------------------------------------------
        def emit_qk(b, ci):
            """S^T tile for chunk ci: cols 0:512 head0, 512:1024 head1."""
            ts = b * TQ
            off = ci * CK - ts
            lo = max(off, 0)
            s = psS.tile([128, 2 * TQ], F32, tag="ps", name=f"s{b}_{ci}")
            for h in range(2):
                nc.tensor.matmul(
                    out=s[:, h * TQ + lo : (h + 1) * TQ],
                    lhsT=kT_sb[h * HD : (h + 1) * HD, ci * CK : (ci + 1) * CK],
                    rhs=qT_sb[h * HD : (h + 1) * HD, ts + lo : ts + TQ],
                    start=True,
                    stop=True,
                )
            return s

        def emit_exp(b, ci, s):
            ts = b * TQ
            off = ci * CK - ts
            p = pwork.tile([128, 2 * TQ], BF16, tag="pt", name=f"p{b}_{ci}")
            if off <= 0:
                nc.scalar.activation(
                    out=p[:], in_=s[:], func=mybir.ActivationFunctionType.Exp,
                    bias=zb_sb[:],
                )
            else:
                # only the causally reachable columns [off:512] of each head
                pr = p[:].rearrange("p (two t) -> p two t", two=2)[:, :, off:TQ]
                sr = s[:].rearrange("p (two t) -> p two t", two=2)[:, :, off:TQ]
                nc.scalar.activation(
                    out=pr, in_=sr, func=mybir.ActivationFunctionType.Exp,
                    bias=zb_sb[:],
                )
            if off >= 0:
                # zero the masked region (incl. stale cols < off):
                # mx[m, n + 512 - off] = 1{n >= m + off}
                for h in range(2):
                    nc.vector.tensor_mul(
                        p[:, h * TQ : (h + 1) * TQ],
                        p[:, h * TQ : (h + 1) * TQ],
                        mx_sb[:, TQ - off : 2 * TQ - off],
                    )
            return p

        def emit_pv(b, ci, p, y0, y1):
            nch = 4 * (b + 1)
            nc.tensor.matmul(
                out=y0[:],
                lhsT=v0_sb[:, ci * VW : (ci + 1) * VW],
                rhs=p[:, 0:TQ],
                start=(ci == 0),
                stop=(ci == nch - 1),
            )
            nc.tensor.matmul(
                out=y1[:],
                lhsT=v1_sb[:, ci * VW : (ci + 1) * VW],
                rhs=p[:, TQ : 2 * TQ],
                start=(ci == 0),
                stop=(ci == nch - 1),
            )

        # p tiles are only partially written on diagonal chunks (exp skips
        # cols the mask zeroes); pre-zero the 3 rotating buffers so stale
        # regions are always finite (NaN * 0 = NaN otherwise).
        for i in range(4):
            pini = pwork.tile([128, 2 * TQ], BF16, tag="pt", name=f"pini{i}")
            nc.vector.memset(pini[:], 0.0)

        ygtiles = {}

        def emit_norm(b):
            # K=1 bf16 matmuls broadcast the denominator rows across
            # partitions, then 1/x and one multiply; ship block to DRAM.
            rbb = psS.tile([128, TQ], F32, tag="ps", name=f"rbb{b}")
            for h in range(2):
                i = 2 * b + h
                pr = (i % 4) * 32
                cr = (i // 4) * TQ
                nc.tensor.matmul(
                    out=rbb[h * HD : (h + 1) * HD, :],
                    lhsT=onesp_sb[pr : pr + 1, 0:HD],
                    rhs=dsp_sb[pr : pr + 1, cr : cr + TQ],
                    start=True,
                    stop=True,
                    tile_position=(pr, h * HD),
                )
            rq = ynp.tile([128, TQ], F32, tag="rq", name=f"rq{b}")
            nc.vector.reciprocal_approx_fast(out=rq[:], in_=rbb[:])
            yn = ynp.tile([128, TQ], BF16, tag="yn", name=f"yn{b}")
            nc.vector.tensor_mul(yn[:], y_sb[:, b * TQ : (b + 1) * TQ], rq[:])
            nc.sync.dma_start(out=sendb[b], in_=yn[:])

        def emit_gather(gi):
            grp = CGRP[gi]
            b0 = grp[0]
            nc.gpsimd.collective_compute(
                "AllGather",
                mybir.AluOpType.bypass,
                replica_groups=[list(range(NCORES))],
                ins=[sendb[b0 : b0 + len(grp)].opt()],
                outs=[gaths[gi].opt()],
            )

        def emit_recv(gi, tail=False, only=None):
            grp = CGRP[gi]
            nsplit = 4 if tail else 2
            eng = nc.gpsimd if tail else nc.sync
            for bi, b in enumerate(grp):
                if only is not None and b not in only:
                    continue
                yg = ygp.tile([128, KD * TQ], BF16, tag="yg", name=f"yg{b}")
                for jh in range(nsplit):
                    j0 = jh * (NCORES // nsplit)
                    eng.dma_start(
                        out=yg[:].rearrange("p (j t) -> p j t", t=TQ)[
                            :, j0 : j0 + NCORES // nsplit
                        ],
                        in_=gaths[gi][j0 : j0 + NCORES // nsplit, bi].rearrange(
                            "j p t -> p j t"
                        ),
                    )
                ygtiles[b] = yg

        def emit_proj(b):
            yg = ygtiles.pop(b)
            po = psS.tile([128, TQ], F32, tag="ps", name=f"po{b}")
            for mt in range(4):
                for j in range(KD):
                    nc.tensor.matmul(
                        out=po[:, mt * 128 : (mt + 1) * 128],
                        lhsT=yg[:, j * TQ + mt * 128 : j * TQ + (mt + 1) * 128],
                        rhs=wp_sb[:, j * E : (j + 1) * E],
                        start=(j == 0),
                        stop=(j == KD - 1),
                    )
            ob = osb.tile([128, TQ], F32, tag="o", name=f"ob{b}")
            nc.vector.tensor_copy(out=ob[:], in_=po[:])
            for mh in range(2):
                nc.sync.dma_start(
                    out=out[b * TQ + mh * 256 : b * TQ + (mh + 1) * 256, :].rearrange(
                        "(mt p) e -> p mt e", p=128
                    ),
                    in_=ob[:].rearrange("p (mt e) -> p mt e", mt=4)[
                        :, mh * 2 : (mh + 1) * 2
                    ],
                )

        def emit_stash(b, y0, y1):
            ts = b * TQ
            for h, yy in ((0, y0), (1, y1)):
                i = 2 * b + h
                nc.vector.tensor_copy(
                    out=dsp_sb[
                        (i % 4) * 32 : (i % 4) * 32 + 1,
                        (i // 4) * TQ : (i // 4 + 1) * TQ,
                    ],
                    in_=yy[HD : HD + 1, :],
                )
            nc.vector.tensor_copy(out=y_sb[0:HD, ts : ts + TQ], in_=y0[0:HD, :])
            nc.vector.tensor_copy(out=y_sb[HD:128, ts : ts + TQ], in_=y1[0:HD, :])

        # ---- main loop ---------------------------------------------------
        emit_q(0)
        emit_k(0)
        for ci in range(4):
            emit_v(ci)

        carry_s, carry_p = {}, {}
        for b in range(NB):
            nch = 4 * (b + 1)
            fillers = []
            if b >= 1:
                fillers.append(lambda bb=b - 1: emit_norm(bb))
            if b == 4:
                fillers.append(lambda: emit_gather(0))
            if b == 5:
                fillers.append(lambda: emit_recv(0))
            if b == 7:
                fillers.append(lambda: emit_gather(1))
            if b + 1 < NB:
                fillers.append(lambda tb=b + 1: emit_q(tb))
                fillers.append(lambda tb=b + 1: emit_k(tb))
                fillers += [lambda c=ci: emit_v(c) for ci in range(4 * b + 4, 4 * b + 8)]
            if b == 6:
                fillers.append(lambda: emit_proj(0))
            if b == 7:
                # only AG0-backed projections (data delivered blocks ago);
                # blocks 4-6 project in the epilogue under the tail mesh wait
                fillers += [lambda bb=bb: emit_proj(bb) for bb in (1, 2, 3)]

            y0 = psY.tile([VW, TQ], F32, tag="py", name=f"y0_{b}")
            y1 = psY.tile([VW, TQ], F32, tag="py", name=f"y1_{b}")
            # software pipeline carried ACROSS blocks: the next block's first
            # two QKs + first exp are emitted in this block's last iterations
            # so the Scalar engine's exp chain never breaks at a boundary.
            s = carry_s
            p = carry_p
            carry_s, carry_p = {}, {}
            if 0 not in p and 0 not in s:
                s[0] = emit_qk(b, 0)
            if 1 not in s:
                s[1] = emit_qk(b, 1)
            if 0 not in p:
                p[0] = emit_exp(b, 0, s.pop(0))
            nf = 0
            for ci in range(nch):
                t2 = ci + 2
                if t2 < nch:
                    s[t2] = emit_qk(b, t2)
                elif b + 1 < NB and t2 - nch < 2:
                    carry_s[t2 - nch] = emit_qk(b + 1, t2 - nch)
                t1 = ci + 1
                if t1 < nch:
                    p[t1] = emit_exp(b, t1, s.pop(t1))
                elif b + 1 < NB and t1 == nch:
                    carry_p[0] = emit_exp(b + 1, 0, carry_s.pop(0))
                # fillers between QK and PV: if PV's dep is late, the PE
                # head-of-line still has ready work; drain one iter early so
                # the block transition isn't a filler dump
                want = ((ci + 1) * len(fillers)) // max(nch - 1, 1)
                while nf < min(want, len(fillers)):
                    fillers[nf]()
                    nf += 1
                emit_pv(b, ci, p.pop(ci), y0, y1)
            emit_stash(b, y0, y1)
            while nf < len(fillers):
                fillers[nf]()
                nf += 1

        emit_norm(NB - 1)
        emit_gather(2)
        emit_recv(1)
        emit_proj(4)
        emit_proj(5)
        emit_proj(6)
        emit_recv(2, tail=True)
        emit_proj(NB - 1)

    nc.compile()
    return nc


def _inputs(x, w_attn, w_proj):
    x = np.asarray(x, dtype=np.float32).reshape(T, D)
    w_attn = np.asarray(w_attn, dtype=np.float32)
    w_proj = np.asarray(w_proj, dtype=np.float32)

    xT_np = np.ascontiguousarray(x.T).astype(NPBF16)
    wpT_np = np.ascontiguousarray(w_proj.T)
    scale = 1.0 / math.sqrt(HD)
    mx_np = (
        np.arange(1024, dtype=np.int32)[None, :]
        >= (np.arange(128, dtype=np.int32)[:, None] + 512)
    ).astype(NPBF16)

    def shuf(wT):  # [D, E] -> SBUF layout [128, KD*E], 2KB-contiguous rows
        return np.ascontiguousarray(
            wT.reshape(KD, 128, E).transpose(1, 0, 2).reshape(128, KD * E)
        ).astype(NPBF16)

    in_maps = []
    for core in range(NCORES):
        r0 = core * E
        in_maps.append(
            {
                "xT": xT_np,
                "wqT": shuf((w_attn[r0 : r0 + E, :] * scale).T),
                "wkT": shuf(w_attn[D + r0 : D + r0 + E, :].T),
                "wvT": shuf(w_attn[2 * D + r0 : 2 * D + r0 + E, :].T),
                "wpT": shuf(wpT_np[:, r0 : r0 + E]),
                "mx": mx_np,
            }
        )
    return in_maps


def kernel(x, w_attn, w_proj, _trace=False):
    if "nc" not in _CACHE:
        _CACHE["nc"] = _build()
    nc = _CACHE["nc"]
    in_maps = _inputs(x, w_attn, w_proj)
    res = run_bass_kernel_spmd(
        nc, in_maps, core_ids=list(range(NCORES)), trace=_trace
    )
    _CACHE["last_result"] = res
    full = np.concatenate([res.results[c]["out"] for c in range(NCORES)], axis=1)
    return full.reshape(B, T, D).astype(np.float32)
